# revision 5
# baseline (speedup 1.0000x reference)
"""CREDA loss kernel for Trainium2 (8 NeuronCores, SPMD data-parallel).

Math: the loss needs only K^2 = exp(-2*d2/(2*sigma^2+EPS)) entries, never K.
With f = 2/(2*sigma^2+EPS):  K2[i,j] = exp(2f*G[i,j]) * exp(-f*r[i]) * exp(-f*r[j]),
G = X @ Y.T.  Every per-class reduction is a quadratic form u^T K2 v, so the
device computes L[c,j] = sum_i u[i,c] * exp(2f*G[i,j] - f*r[i])  (GEMM ->
ScalarE exp with per-partition bias -> [128,4]x[128,512] reduction matmul into
PSUM), and the host folds exp(-f*r[j]) plus the right-hand mask into a tiny dot.

Sharding: each core owns a 512-row slice of all three blocks (ss, tt, st) and
streams the full 4096-column side; per-core partial L[3,4,4096] summed on host.
"""

import numpy as np
import ml_dtypes

import concourse.bacc as bacc
import concourse.tile as tile
import concourse.mybir as mybir
from concourse.bass_utils import run_bass_kernel_spmd

# Problem constants (hardcoded per harness contract)
N = 4096            # N_S == N_T
D = 1024
C = 4
SIGMA = 32.0
EPS = 1e-8
LOG2 = float(np.log(2.0))
LAMBDA_CREDA = 1.0
LAMBDA_ENTROPY = 0.1

NCORES = 8
ROWS = N // NCORES      # 512 rows per core
IT = 128                # i-tile (PSUM partition dim)
JT = 512                # j-tile (PSUM free dim, one bank fp32)
KC = 128                # contraction chunk (PE partition dim)
N_I = ROWS // IT        # 4
N_J = N // JT           # 8
N_K = D // KC           # 8

F_SCALE = 2.0 / (2.0 * SIGMA * SIGMA + EPS)   # exponent factor for K^2
ACT_SCALE = float(2.0 * F_SCALE)              # multiplies G inside the exp

BF16 = mybir.dt.bfloat16
FP16 = mybir.dt.float16
FP32 = mybir.dt.float32

_COMPILED = {}


def _build(repeat=1):
    nc = bacc.Bacc("TRN2", target_bir_lowering=False, debug=False)

    rhs_s = nc.dram_tensor("rhs_s", [D, N], BF16, kind="ExternalInput")
    rhs_t = nc.dram_tensor("rhs_t", [D, N], BF16, kind="ExternalInput")
    lhs_s = nc.dram_tensor("lhs_s", [D, ROWS], BF16, kind="ExternalInput")
    lhs_t = nc.dram_tensor("lhs_t", [D, ROWS], BF16, kind="ExternalInput")
    lm_s = nc.dram_tensor("lm_s", [IT, N_I, C], FP16, kind="ExternalInput")
    lm_t = nc.dram_tensor("lm_t", [IT, N_I, C], FP16, kind="ExternalInput")
    bias_s = nc.dram_tensor("bias_s", [IT, N_I], FP32, kind="ExternalInput")
    bias_t = nc.dram_tensor("bias_t", [IT, N_I], FP32, kind="ExternalInput")
    lout = nc.dram_tensor("lout", [3, C, N], FP32, kind="ExternalOutput")

    rs_ap = rhs_s.ap().rearrange("(k p) j -> p k j", p=KC)
    rt_ap = rhs_t.ap().rearrange("(k p) j -> p k j", p=KC)

    with tile.TileContext(nc) as tc:
        with (
            tc.tile_pool(name="const", bufs=1) as const,
            tc.tile_pool(name="rhs", bufs=2) as rhsp,
            tc.tile_pool(name="ep", bufs=4) as epp,
            tc.tile_pool(name="stage", bufs=2) as stp,
            tc.tile_pool(name="gps", bufs=2, space="PSUM") as gps,
            tc.tile_pool(name="lps", bufs=2, space="PSUM") as lps,
        ):
            lhsS = const.tile([KC, N_K, ROWS], BF16)
            nc.sync.dma_start(out=lhsS, in_=lhs_s.ap().rearrange("(k p) i -> p k i", p=KC))
            lhsT = const.tile([KC, N_K, ROWS], BF16)
            nc.sync.dma_start(out=lhsT, in_=lhs_t.ap().rearrange("(k p) i -> p k i", p=KC))
            lmS = const.tile([IT, N_I, C], FP16)
            nc.sync.dma_start(out=lmS, in_=lm_s.ap())
            lmT = const.tile([IT, N_I, C], FP16)
            nc.sync.dma_start(out=lmT, in_=lm_t.ap())
            biS = const.tile([IT, N_I], FP32)
            nc.sync.dma_start(out=biS, in_=bias_s.ap())
            biT = const.tile([IT, N_I], FP32)
            nc.sync.dma_start(out=biT, in_=bias_t.ap())

            def body():
                for j in range(N_J):
                    _emit_jtile(nc, tc, j, rs_ap, rt_ap, rhsp, epp, stp, gps, lps,
                                lhsS, lhsT, lmS, lmT, biS, biT, lout)

            if repeat == 1:
                body()
            else:
                with tc.For_i(0, repeat, 1):
                    body()

    nc.compile()
    return nc


def _emit_jtile(nc, tc, j, rs_ap, rt_ap, rhsp, epp, stp, gps, lps,
                lhsS, lhsT, lmS, lmT, biS, biT, lout):
    rS = rhsp.tile([KC, N_K, JT], BF16, tag="rS")
    nc.sync.dma_start(out=rS, in_=rs_ap[:, :, j * JT:(j + 1) * JT])
    rT = rhsp.tile([KC, N_K, JT], BF16, tag="rT")
    nc.sync.dma_start(out=rT, in_=rt_ap[:, :, j * JT:(j + 1) * JT])

    blocks = (
        (lhsS, rS, lmS, biS),   # ss
        (lhsT, rT, lmT, biT),   # tt
        (lhsS, rT, lmS, biS),   # st
    )
    for b, (lhs, rhs, lm, bi) in enumerate(blocks):
        lp = lps.tile([C, JT], FP32)
        for it in range(N_I):
            gp = gps.tile([IT, JT], FP32)
            for k in range(N_K):
                nc.tensor.matmul(
                    gp,
                    lhs[:, k, it * IT:(it + 1) * IT],
                    rhs[:, k, :],
                    start=(k == 0),
                    stop=(k == N_K - 1),
                )
            ep = epp.tile([IT, JT], FP16)
            nc.scalar.activation(
                ep, gp, mybir.ActivationFunctionType.Exp,
                bias=bi[:, it:it + 1], scale=ACT_SCALE,
            )
            nc.tensor.matmul(
                lp, lm[:, it, :], ep,
                start=(it == 0), stop=(it == N_I - 1),
            )
        st = stp.tile([C, JT], FP32)
        nc.vector.tensor_copy(st, lp)
        nc.sync.dma_start(out=lout.ap()[b, :, j * JT:(j + 1) * JT], in_=st)


def _get_nc(repeat=1):
    if repeat not in _COMPILED:
        _COMPILED[repeat] = _build(repeat)
    return _COMPILED[repeat]


def _host_prep(features_s, logits_s, features_t, logits_t, labels_s):
    fs = np.asarray(features_s, dtype=np.float32)
    ft = np.asarray(features_t, dtype=np.float32)
    lt = np.asarray(logits_t, dtype=np.float32)
    lab = np.asarray(labels_s).astype(np.int64)

    # target softmax / pseudo labels / uncertainty weights (host, fp64)
    z = lt.astype(np.float64)
    z = z - z.max(axis=1, keepdims=True)
    pt = np.exp(z)
    pt /= pt.sum(axis=1, keepdims=True)
    pseudo = np.argmax(pt, axis=1)
    h2p = -np.log(np.sum(pt * pt, axis=1) + EPS) / LOG2
    h2max = np.log(float(C)) / LOG2
    w = 1.0 - h2p / (h2max + EPS)

    ms = np.zeros((N, C), dtype=np.float64)
    ms[np.arange(N), lab] = 1.0
    mt = np.zeros((N, C), dtype=np.float64)
    mt[np.arange(N), pseudo] = 1.0
    wt2 = mt * (w * w)[:, None]          # [N, C] == Wt2.T of the reference

    rs = np.sum(fs.astype(np.float64) ** 2, axis=1)
    rt = np.sum(ft.astype(np.float64) ** 2, axis=1)
    es = np.exp(-F_SCALE * rs)
    et = np.exp(-F_SCALE * rt)

    xsT = np.ascontiguousarray(fs.T).astype(ml_dtypes.bfloat16)
    xtT = np.ascontiguousarray(ft.T).astype(ml_dtypes.bfloat16)

    in_maps = []
    for c in range(NCORES):
        sl = slice(c * ROWS, (c + 1) * ROWS)
        lm_s_c = ms[sl].reshape(N_I, IT, C).transpose(1, 0, 2)
        lm_t_c = wt2[sl].reshape(N_I, IT, C).transpose(1, 0, 2)
        bi_s_c = (-F_SCALE * rs[sl]).reshape(N_I, IT).T
        bi_t_c = (-F_SCALE * rt[sl]).reshape(N_I, IT).T
        in_maps.append({
            "rhs_s": xsT,
            "rhs_t": xtT,
            "lhs_s": np.ascontiguousarray(xsT[:, sl]),
            "lhs_t": np.ascontiguousarray(xtT[:, sl]),
            "lm_s": np.ascontiguousarray(lm_s_c).astype(np.float16),
            "lm_t": np.ascontiguousarray(lm_t_c).astype(np.float16),
            "bias_s": np.ascontiguousarray(bi_s_c).astype(np.float32),
            "bias_t": np.ascontiguousarray(bi_t_c).astype(np.float32),
        })

    aux = dict(ms=ms, mt=mt, wt2=wt2, es=es, et=et, w=w, lab=lab, pt=pt)
    return in_maps, aux


def _host_finish(L, aux, logits_s):
    ms, mt, wt2 = aux["ms"], aux["mt"], aux["wt2"]
    es, et, lab, pt = aux["es"], aux["et"], aux["lab"], aux["pt"]

    # right-hand side masks with the column exp factor folded in
    rm_ss = ms * es[:, None]             # [N, C]
    rm_tt = wt2 * et[:, None]
    rm_st = mt * et[:, None]

    ss_s = np.einsum("cj,jc->c", L[0], rm_ss)
    ss_t = np.einsum("cj,jc->c", L[1], rm_tt)
    ss_st = np.einsum("cj,jc->c", L[2], rm_st)

    n_s = ms.sum(axis=0)
    n_t = mt.sum(axis=0)
    tr_s = n_s
    tr_t = wt2.sum(axis=0)

    def h2(tr, sumsq):
        info = sumsq / (tr + EPS) ** 2
        return -np.log(info + EPS) / LOG2

    h_s = h2(tr_s, ss_s)
    h_t = h2(tr_t, ss_t)
    h_mix = h2(tr_s + tr_t, ss_s + 2.0 * ss_st + ss_t)
    per_class = h_mix - 0.5 * (h_s + h_t)
    valid = (n_s >= 2) & (n_t >= 2)
    n_valid = float(valid.sum())
    creda_sum = float(np.where(valid, per_class, 0.0).sum())
    loss_creda = creda_sum / max(n_valid, 1.0) if n_valid > 0 else 0.0

    # source cross entropy (host, fp64)
    zs = np.asarray(logits_s, dtype=np.float64)
    zs = zs - zs.max(axis=1, keepdims=True)
    lse = np.log(np.exp(zs).sum(axis=1))
    logp = zs - lse[:, None]
    loss_cls = -float(np.mean(logp[np.arange(N), lab]))

    # target entropy
    loss_ent = -float(np.mean(np.sum(pt * np.log(pt + EPS), axis=1)))

    total = loss_cls + LAMBDA_CREDA * loss_creda + LAMBDA_ENTROPY * loss_ent
    return np.array(total, dtype=np.float32)


def run(inputs, trace=False, repeat=1):
    """Full pipeline; returns (loss, BassKernelResults)."""
    in_maps, aux = _host_prep(**inputs)
    nc = _get_nc(repeat)
    res = run_bass_kernel_spmd(
        nc, in_maps, core_ids=list(range(NCORES)), trace=trace,
    )
    L = np.zeros((3, C, N), dtype=np.float64)
    for r in res.results:
        L += r["lout"].astype(np.float64)
    loss = _host_finish(L, aux, inputs["logits_s"])
    return loss, res


def kernel(**inputs) -> np.ndarray:
    loss, _ = run(inputs, trace=False)
    return loss


# revision 6
# speedup vs baseline: 1.0985x; 1.0985x over previous
"""CREDA loss kernel for Trainium2 (8 NeuronCores, SPMD data-parallel).

Math: the loss needs only K^2 = exp(-2*d2/(2*sigma^2+EPS)) entries, never K.
With f = 2/(2*sigma^2+EPS):  K2[i,j] = exp(2f*G[i,j]) * exp(-f*r[i]) * exp(-f*r[j]),
G = X @ Y.T.  Every per-class reduction is a quadratic form u^T K2 v, so the
device computes L[c,j] = sum_i u[i,c] * exp(2f*G[i,j] - f*r[i])  (GEMM ->
ScalarE exp with per-partition bias -> [128,4]x[128,512] reduction matmul into
PSUM), and the host folds exp(-f*r[j]) plus the right-hand mask into a tiny dot.

Sharding: each core owns a 512-row slice of all three blocks (ss, tt, st) and
streams the full 4096-column side; per-core partial L[3,4,4096] summed on host.
"""

import numpy as np
import ml_dtypes

import concourse.bacc as bacc
import concourse.tile as tile
import concourse.mybir as mybir
from concourse.bass_utils import run_bass_kernel_spmd

# Problem constants (hardcoded per harness contract)
N = 4096            # N_S == N_T
D = 1024
C = 4
SIGMA = 32.0
EPS = 1e-8
LOG2 = float(np.log(2.0))
LAMBDA_CREDA = 1.0
LAMBDA_ENTROPY = 0.1

NCORES = 8
ROWS = N // NCORES      # 512 rows per core
IT = 128                # i-tile (PSUM partition dim)
JT = 512                # j-tile (PSUM free dim, one bank fp32)
KC = 128                # contraction chunk (PE partition dim)
N_I = ROWS // IT        # 4
N_J = N // JT           # 8
N_K = D // KC           # 8

F_SCALE = 2.0 / (2.0 * SIGMA * SIGMA + EPS)   # exponent factor for K^2
ACT_SCALE = float(2.0 * F_SCALE)              # multiplies G inside the exp

BF16 = mybir.dt.bfloat16
FP16 = mybir.dt.float16
FP32 = mybir.dt.float32

_COMPILED = {}


def _build(repeat=1):
    nc = bacc.Bacc("TRN2", target_bir_lowering=False, debug=False)

    rhs_s = nc.dram_tensor("rhs_s", [D, N], BF16, kind="ExternalInput")
    rhs_t = nc.dram_tensor("rhs_t", [D, N], BF16, kind="ExternalInput")
    lhs_s = nc.dram_tensor("lhs_s", [D, ROWS], BF16, kind="ExternalInput")
    lhs_t = nc.dram_tensor("lhs_t", [D, ROWS], BF16, kind="ExternalInput")
    lm_s = nc.dram_tensor("lm_s", [IT, N_I, C], FP16, kind="ExternalInput")
    lm_t = nc.dram_tensor("lm_t", [IT, N_I, C], FP16, kind="ExternalInput")
    bias_s = nc.dram_tensor("bias_s", [IT, N_I], FP32, kind="ExternalInput")
    bias_t = nc.dram_tensor("bias_t", [IT, N_I], FP32, kind="ExternalInput")
    lout = nc.dram_tensor("lout", [3, C, N], FP32, kind="ExternalOutput")

    rs_ap = rhs_s.ap().rearrange("(k p) j -> p k j", p=KC)
    rt_ap = rhs_t.ap().rearrange("(k p) j -> p k j", p=KC)

    with tile.TileContext(nc) as tc:
        with (
            tc.tile_pool(name="const", bufs=1) as const,
            tc.tile_pool(name="ep", bufs=6) as epp,
            tc.tile_pool(name="stage", bufs=2) as stp,
            tc.tile_pool(name="gps", bufs=3, space="PSUM") as gps,
            tc.tile_pool(name="lps", bufs=2, space="PSUM") as lps,
        ):
            lhsS = const.tile([KC, N_K, ROWS], BF16)
            nc.sync.dma_start(out=lhsS, in_=lhs_s.ap().rearrange("(k p) i -> p k i", p=KC))
            lhsT = const.tile([KC, N_K, ROWS], BF16)
            nc.sync.dma_start(out=lhsT, in_=lhs_t.ap().rearrange("(k p) i -> p k i", p=KC))
            lmS = const.tile([IT, N_I, C], FP16)
            nc.sync.dma_start(out=lmS, in_=lm_s.ap())
            lmT = const.tile([IT, N_I, C], FP16)
            nc.sync.dma_start(out=lmT, in_=lm_t.ap())
            biS = const.tile([IT, N_I], FP32)
            nc.sync.dma_start(out=biS, in_=bias_s.ap())
            biT = const.tile([IT, N_I], FP32)
            nc.sync.dma_start(out=biT, in_=bias_t.ap())

            # both rhs matrices fully SBUF-resident, one tile per j-panel so
            # consumers gate on individual panel DMAs (16 x 1MB)
            rsP = []
            rtP = []
            for j in range(N_J):
                rs = const.tile([KC, N_K, JT], BF16, tag=f"rs{j}")
                nc.sync.dma_start(out=rs, in_=rs_ap[:, :, j * JT:(j + 1) * JT])
                rsP.append(rs)
                rt = const.tile([KC, N_K, JT], BF16, tag=f"rt{j}")
                nc.sync.dma_start(out=rt, in_=rt_ap[:, :, j * JT:(j + 1) * JT])
                rtP.append(rt)

            def body():
                for j in range(N_J):
                    _emit_jtile(nc, tc, j, rsP[j], rtP[j], epp, stp, gps, lps,
                                lhsS, lhsT, lmS, lmT, biS, biT, lout)

            if repeat == 1:
                body()
            else:
                with tc.For_i(0, repeat, 1, staggered_reset=True):
                    body()

    nc.compile()
    return nc


def _emit_jtile(nc, tc, j, rS, rT, epp, stp, gps, lps,
                lhsS, lhsT, lmS, lmT, biS, biT, lout):
    blocks = (
        (lhsS, rS, lmS, biS),   # ss
        (lhsT, rT, lmT, biT),   # tt
        (lhsS, rT, lmS, biS),   # st
    )
    for b, (lhs, rhs, lm, bi) in enumerate(blocks):
        lp = lps.tile([C, JT], FP32)
        # software pipeline: emit the reduce-matmul for i-tile `it` after the
        # G matmuls of i-tile `it+1`, so PE never waits on ACT's exp
        pend = None
        for it in range(N_I):
            gp = gps.tile([IT, JT], FP32)
            for k in range(N_K):
                nc.tensor.matmul(
                    gp,
                    lhs[:, k, it * IT:(it + 1) * IT],
                    rhs[:, k, :],
                    start=(k == 0),
                    stop=(k == N_K - 1),
                )
            ep = epp.tile([IT, JT], FP16)
            nc.scalar.activation(
                ep, gp, mybir.ActivationFunctionType.Exp,
                bias=bi[:, it:it + 1], scale=ACT_SCALE,
            )
            if pend is not None:
                nc.tensor.matmul(
                    lp, lm[:, pend[0], :], pend[1],
                    start=(pend[0] == 0), stop=False,
                    skip_group_check=True,
                )
            pend = (it, ep)
        nc.tensor.matmul(
            lp, lm[:, pend[0], :], pend[1],
            start=False, stop=True,
            skip_group_check=True,
        )
        st = stp.tile([C, JT], FP32)
        nc.vector.tensor_copy(st, lp)
        nc.sync.dma_start(out=lout.ap()[b, :, j * JT:(j + 1) * JT], in_=st)


def _get_nc(repeat=1):
    if repeat not in _COMPILED:
        _COMPILED[repeat] = _build(repeat)
    return _COMPILED[repeat]


def _host_prep(features_s, logits_s, features_t, logits_t, labels_s):
    fs = np.asarray(features_s, dtype=np.float32)
    ft = np.asarray(features_t, dtype=np.float32)
    lt = np.asarray(logits_t, dtype=np.float32)
    lab = np.asarray(labels_s).astype(np.int64)

    # target softmax / pseudo labels / uncertainty weights (host, fp64)
    z = lt.astype(np.float64)
    z = z - z.max(axis=1, keepdims=True)
    pt = np.exp(z)
    pt /= pt.sum(axis=1, keepdims=True)
    pseudo = np.argmax(pt, axis=1)
    h2p = -np.log(np.sum(pt * pt, axis=1) + EPS) / LOG2
    h2max = np.log(float(C)) / LOG2
    w = 1.0 - h2p / (h2max + EPS)

    ms = np.zeros((N, C), dtype=np.float64)
    ms[np.arange(N), lab] = 1.0
    mt = np.zeros((N, C), dtype=np.float64)
    mt[np.arange(N), pseudo] = 1.0
    wt2 = mt * (w * w)[:, None]          # [N, C] == Wt2.T of the reference

    rs = np.sum(fs.astype(np.float64) ** 2, axis=1)
    rt = np.sum(ft.astype(np.float64) ** 2, axis=1)
    es = np.exp(-F_SCALE * rs)
    et = np.exp(-F_SCALE * rt)

    xsT = np.ascontiguousarray(fs.T).astype(ml_dtypes.bfloat16)
    xtT = np.ascontiguousarray(ft.T).astype(ml_dtypes.bfloat16)

    in_maps = []
    for c in range(NCORES):
        sl = slice(c * ROWS, (c + 1) * ROWS)
        lm_s_c = ms[sl].reshape(N_I, IT, C).transpose(1, 0, 2)
        lm_t_c = wt2[sl].reshape(N_I, IT, C).transpose(1, 0, 2)
        bi_s_c = (-F_SCALE * rs[sl]).reshape(N_I, IT).T
        bi_t_c = (-F_SCALE * rt[sl]).reshape(N_I, IT).T
        in_maps.append({
            "rhs_s": xsT,
            "rhs_t": xtT,
            "lhs_s": np.ascontiguousarray(xsT[:, sl]),
            "lhs_t": np.ascontiguousarray(xtT[:, sl]),
            "lm_s": np.ascontiguousarray(lm_s_c).astype(np.float16),
            "lm_t": np.ascontiguousarray(lm_t_c).astype(np.float16),
            "bias_s": np.ascontiguousarray(bi_s_c).astype(np.float32),
            "bias_t": np.ascontiguousarray(bi_t_c).astype(np.float32),
        })

    aux = dict(ms=ms, mt=mt, wt2=wt2, es=es, et=et, w=w, lab=lab, pt=pt)
    return in_maps, aux


def _host_finish(L, aux, logits_s):
    ms, mt, wt2 = aux["ms"], aux["mt"], aux["wt2"]
    es, et, lab, pt = aux["es"], aux["et"], aux["lab"], aux["pt"]

    # right-hand side masks with the column exp factor folded in
    rm_ss = ms * es[:, None]             # [N, C]
    rm_tt = wt2 * et[:, None]
    rm_st = mt * et[:, None]

    ss_s = np.einsum("cj,jc->c", L[0], rm_ss)
    ss_t = np.einsum("cj,jc->c", L[1], rm_tt)
    ss_st = np.einsum("cj,jc->c", L[2], rm_st)

    n_s = ms.sum(axis=0)
    n_t = mt.sum(axis=0)
    tr_s = n_s
    tr_t = wt2.sum(axis=0)

    def h2(tr, sumsq):
        info = sumsq / (tr + EPS) ** 2
        return -np.log(info + EPS) / LOG2

    h_s = h2(tr_s, ss_s)
    h_t = h2(tr_t, ss_t)
    h_mix = h2(tr_s + tr_t, ss_s + 2.0 * ss_st + ss_t)
    per_class = h_mix - 0.5 * (h_s + h_t)
    valid = (n_s >= 2) & (n_t >= 2)
    n_valid = float(valid.sum())
    creda_sum = float(np.where(valid, per_class, 0.0).sum())
    loss_creda = creda_sum / max(n_valid, 1.0) if n_valid > 0 else 0.0

    # source cross entropy (host, fp64)
    zs = np.asarray(logits_s, dtype=np.float64)
    zs = zs - zs.max(axis=1, keepdims=True)
    lse = np.log(np.exp(zs).sum(axis=1))
    logp = zs - lse[:, None]
    loss_cls = -float(np.mean(logp[np.arange(N), lab]))

    # target entropy
    loss_ent = -float(np.mean(np.sum(pt * np.log(pt + EPS), axis=1)))

    total = loss_cls + LAMBDA_CREDA * loss_creda + LAMBDA_ENTROPY * loss_ent
    return np.array(total, dtype=np.float32)


def run(inputs, trace=False, repeat=1):
    """Full pipeline; returns (loss, BassKernelResults)."""
    in_maps, aux = _host_prep(**inputs)
    nc = _get_nc(repeat)
    res = run_bass_kernel_spmd(
        nc, in_maps, core_ids=list(range(NCORES)), trace=trace,
    )
    L = np.zeros((3, C, N), dtype=np.float64)
    for r in res.results:
        L += r["lout"].astype(np.float64)
    loss = _host_finish(L, aux, inputs["logits_s"])
    return loss, res


def kernel(**inputs) -> np.ndarray:
    loss, _ = run(inputs, trace=False)
    return loss


# revision 47
# speedup vs baseline: 2.4688x; 2.2475x over previous
"""CREDA loss kernel for Trainium2 (8 NeuronCores, SPMD data-parallel).

Math: the loss needs only K^2 = exp(-2*d2/(2*sigma^2+EPS)) entries, never K.
With f = 2/(2*sigma^2+EPS):  K2[i,j] = exp(2f*G[i,j]) * exp(-f*r[i]) * exp(-f*r[j]),
G = X @ Y.T.  Every per-class reduction is a quadratic form u^T K2 v, so the
device computes L[c,j] = sum_i u[i,c] * exp(2f*G[i,j] + bias_i)  (fp8 DoubleRow
GEMM -> ScalarE exp with per-partition fp32 bias -> [128,4]x[128,512] bf16
reduction matmul into PSUM), and the host folds exp(-f*r[j]) plus the
right-hand class mask into a tiny dot.  The tt block's uncertainty weights
w_i^2 ride the fp32 bias (+2 ln w_i), so all device-side masks are exact 0/1.

Sharding: each core owns a 512-row slice of all three blocks (ss, tt, st);
both feature matrices live SBUF-resident (fp8, 8 MB); per-core partial
L[3,4,4096] outputs are summed on host.  No collectives.
"""

import numpy as np
import ml_dtypes

import concourse.bacc as bacc
import concourse.tile as tile
import concourse.mybir as mybir
from concourse.bass_utils import run_bass_kernel_spmd

# Problem constants (hardcoded per harness contract)
N = 4096            # N_S == N_T
D = 1024
C = 4
SIGMA = 32.0
EPS = 1e-8
LOG2 = float(np.log(2.0))
LAMBDA_CREDA = 1.0
LAMBDA_ENTROPY = 0.1

NCORES = 8
ROWS = N // NCORES      # 512 rows per core
IT = 128                # i-tile (PSUM partition dim)
JT = 512                # j-tile (PSUM free dim, one fp32 bank)
KC = 128                # contraction chunk (PE partition dim)
N_I = ROWS // IT        # 4
N_J = N // JT           # 8
N_K = D // KC           # 8
TT_LIFT = 32.0          # lifts tt exp values; exactly undone on host

F_SCALE = 2.0 / (2.0 * SIGMA * SIGMA + EPS)   # exponent factor for K^2
ACT_SCALE = float(2.0 * F_SCALE)              # multiplies G inside the exp

BF16 = mybir.dt.bfloat16
FP32 = mybir.dt.float32
FP8 = mybir.dt.float8e4

_COMPILED = {}


def _build(repeat=1):
    nc = bacc.Bacc("TRN2", target_bir_lowering=False, debug=False)

    rhs_s = nc.dram_tensor("rhs_s", [D, N], FP8, kind="ExternalInput")
    rhs_t = nc.dram_tensor("rhs_t", [D, N], FP8, kind="ExternalInput")
    lhs_s = nc.dram_tensor("lhs_s", [D, ROWS], FP8, kind="ExternalInput")
    lhs_t = nc.dram_tensor("lhs_t", [D, ROWS], FP8, kind="ExternalInput")
    lm_s = nc.dram_tensor("lm_s", [IT, N_I, C], BF16, kind="ExternalInput")
    lm_t = nc.dram_tensor("lm_t", [IT, N_I, C], BF16, kind="ExternalInput")
    bias_s = nc.dram_tensor("bias_s", [IT, N_I], FP32, kind="ExternalInput")
    bias_t = nc.dram_tensor("bias_t", [IT, N_I], FP32, kind="ExternalInput")
    lout = nc.dram_tensor("lout", [3, C, N], FP32, kind="ExternalOutput")

    rs_ap = rhs_s.ap().rearrange("(k p) j -> p k j", p=KC)
    rt_ap = rhs_t.ap().rearrange("(k p) j -> p k j", p=KC)

    with tile.TileContext(nc) as tc:
        with (
            tc.tile_pool(name="const", bufs=1) as const,
            tc.tile_pool(name="ep", bufs=6) as epp,
            tc.tile_pool(name="stage", bufs=2) as stp,
            tc.tile_pool(name="gps", bufs=2, space="PSUM") as gps,
            tc.tile_pool(name="lps", bufs=2, space="PSUM") as lps,
        ):
            lhsS = const.tile([KC, N_K, ROWS], FP8)
            nc.sync.dma_start(out=lhsS, in_=lhs_s.ap().rearrange("(k p) i -> p k i", p=KC))
            lhsT = const.tile([KC, N_K, ROWS], FP8)
            nc.sync.dma_start(out=lhsT, in_=lhs_t.ap().rearrange("(k p) i -> p k i", p=KC))
            lmS = const.tile([IT, N_I, C], BF16)
            nc.sync.dma_start(out=lmS, in_=lm_s.ap())
            lmT = const.tile([IT, N_I, C], BF16)
            nc.sync.dma_start(out=lmT, in_=lm_t.ap())
            biS = const.tile([IT, N_I], FP32)
            nc.sync.dma_start(out=biS, in_=bias_s.ap())
            biT = const.tile([IT, N_I], FP32)
            nc.sync.dma_start(out=biT, in_=bias_t.ap())

            # both rhs matrices fully SBUF-resident, one tile per j-panel so
            # consumers gate on individual panel DMAs (16 x 0.5 MB)
            rsP = []
            rtP = []
            for j in range(N_J):
                rs = const.tile([KC, N_K, JT], FP8, tag=f"rs{j}")
                nc.sync.dma_start(out=rs, in_=rs_ap[:, :, j * JT:(j + 1) * JT])
                rsP.append(rs)
                rt = const.tile([KC, N_K, JT], FP8, tag=f"rt{j}")
                nc.sync.dma_start(out=rt, in_=rt_ap[:, :, j * JT:(j + 1) * JT])
                rtP.append(rt)

            blocks = (
                (lhsS, rsP, lmS, biS),   # ss
                (lhsT, rtP, lmT, biT),   # tt
                (lhsS, rtP, lmS, biS),   # st
            )

            def body():
                # two j-tiles per pass share one 2-bank PSUM tile and one exp
                for jp in range(N_J // 2):
                    for b in range(3):
                        lhs, rP, lm, bi = blocks[b]
                        j0, j1 = 2 * jp, 2 * jp + 1
                        eps = []
                        for it in range(N_I):
                            gp = gps.tile([IT, 2 * JT], FP32, tag="gp", bufs=2)
                            _g_group(nc, gp[:, 0:JT], lhs, rP[j0], it)
                            _g_group(nc, gp[:, JT:2 * JT], lhs, rP[j1], it)
                            ep = epp.tile([IT, 2 * JT], BF16,
                                          tag=f"ep{it}", bufs=2)
                            nc.scalar.activation(
                                ep, gp, mybir.ActivationFunctionType.Exp,
                                bias=bi[:, it:it + 1], scale=ACT_SCALE,
                            )
                            eps.append(ep)
                        _emit_reduce(nc, stp, lps, lout, lm, eps, b, j0, j1)

            if repeat == 1:
                body()
            else:
                with tc.For_i(0, repeat, 1):
                    body()

    nc.compile()
    return nc


def _g_group(nc, gp, lhs, rhs, it):
    """G = X_i . Y_j over the 1024-d contraction (fp8 DoubleRow, K=256/MM)."""
    for k2 in range(N_K // 2):
        nc.tensor.matmul(
            gp,
            lhs[:, 2 * k2:2 * k2 + 2, it * IT:(it + 1) * IT],
            rhs[:, 2 * k2:2 * k2 + 2, :],
            start=(k2 == 0),
            stop=(k2 == N_K // 2 - 1),
            perf_mode=mybir.MatmulPerfMode.DoubleRow,
        )


def _emit_reduce(nc, stp, lps, lout, lm, eps, b, j0, j1):
    lp0 = lps.tile([C, JT], FP32, tag="lp0", bufs=2, name=f"lp0_{b}_{j0}")
    lp1 = lps.tile([C, JT], FP32, tag="lp1", bufs=2, name=f"lp1_{b}_{j0}")
    for jh, lp in ((0, lp0), (1, lp1)):
        for it in range(N_I):
            nc.tensor.matmul(
                lp, lm[:, it, :], eps[it][:, jh * JT:(jh + 1) * JT],
                start=(it == 0), stop=(it == N_I - 1),
            )
    st = stp.tile([C, 2 * JT], FP32, name=f"st_{b}_{j0}")
    nc.vector.tensor_copy(st[:, 0:JT], lp0)
    nc.vector.tensor_copy(st[:, JT:2 * JT], lp1)
    nc.sync.dma_start(out=lout.ap()[b, :, j0 * JT:(j1 + 1) * JT], in_=st)


def _get_nc(repeat=1):
    if repeat not in _COMPILED:
        _COMPILED[repeat] = _build(repeat)
    return _COMPILED[repeat]


def _host_prep(features_s, logits_s, features_t, logits_t, labels_s):
    fs = np.asarray(features_s, dtype=np.float32)
    ft = np.asarray(features_t, dtype=np.float32)
    lt = np.asarray(logits_t, dtype=np.float32)
    lab = np.asarray(labels_s).astype(np.int64)

    # target softmax / pseudo labels / uncertainty weights (host, fp64)
    z = lt.astype(np.float64)
    z = z - z.max(axis=1, keepdims=True)
    pt = np.exp(z)
    pt /= pt.sum(axis=1, keepdims=True)
    pseudo = np.argmax(pt, axis=1)
    h2p = -np.log(np.sum(pt * pt, axis=1) + EPS) / LOG2
    h2max = np.log(float(C)) / LOG2
    w = 1.0 - h2p / (h2max + EPS)

    ms = np.zeros((N, C), dtype=np.float64)
    ms[np.arange(N), lab] = 1.0
    mt = np.zeros((N, C), dtype=np.float64)
    mt[np.arange(N), pseudo] = 1.0
    wt2 = mt * (w * w)[:, None]          # [N, C] == Wt2.T of the reference

    rs = np.sum(fs.astype(np.float64) ** 2, axis=1)
    rt = np.sum(ft.astype(np.float64) ** 2, axis=1)
    es = np.exp(-F_SCALE * rs)
    et = np.exp(-F_SCALE * rt)

    # tt-block row weights w^2 (and a xTT_LIFT lift) ride the fp32 ACT bias:
    # exp(2fG - f*rt_i + 2 ln w_i + ln TT_LIFT)
    logw2 = np.where(w > 0, 2.0 * np.log(np.maximum(w, 1e-300)), -1e30)
    bias_tt = -F_SCALE * rt + logw2 + np.log(TT_LIFT)

    xsT = np.ascontiguousarray(fs.T).astype(ml_dtypes.float8_e4m3)
    xtT = np.ascontiguousarray(ft.T).astype(ml_dtypes.float8_e4m3)

    in_maps = []
    for c in range(NCORES):
        sl = slice(c * ROWS, (c + 1) * ROWS)
        lm_s_c = ms[sl].reshape(N_I, IT, C).transpose(1, 0, 2)
        lm_t_c = mt[sl].reshape(N_I, IT, C).transpose(1, 0, 2)
        bi_s_c = (-F_SCALE * rs[sl]).reshape(N_I, IT).T
        bi_t_c = bias_tt[sl].reshape(N_I, IT).T
        in_maps.append({
            "rhs_s": xsT,
            "rhs_t": xtT,
            "lhs_s": np.ascontiguousarray(xsT[:, sl]),
            "lhs_t": np.ascontiguousarray(xtT[:, sl]),
            "lm_s": np.ascontiguousarray(lm_s_c).astype(ml_dtypes.bfloat16),
            "lm_t": np.ascontiguousarray(lm_t_c).astype(ml_dtypes.bfloat16),
            "bias_s": np.ascontiguousarray(bi_s_c).astype(np.float32),
            "bias_t": np.ascontiguousarray(bi_t_c).astype(np.float32),
        })

    aux = dict(ms=ms, mt=mt, wt2=wt2, es=es, et=et, w=w, lab=lab, pt=pt)
    return in_maps, aux


def _host_finish(L, aux, logits_s):
    ms, mt, wt2 = aux["ms"], aux["mt"], aux["wt2"]
    es, et, lab, pt = aux["es"], aux["et"], aux["lab"], aux["pt"]

    # right-hand side masks with the column exp factor folded in
    rm_ss = ms * es[:, None]             # [N, C]
    rm_tt = wt2 * et[:, None]
    rm_st = mt * et[:, None]

    ss_s = np.einsum("cj,jc->c", L[0], rm_ss)
    ss_t = np.einsum("cj,jc->c", L[1], rm_tt) / TT_LIFT
    ss_st = np.einsum("cj,jc->c", L[2], rm_st)

    n_s = ms.sum(axis=0)
    n_t = mt.sum(axis=0)
    tr_s = n_s
    tr_t = wt2.sum(axis=0)

    def h2(tr, sumsq):
        info = sumsq / (tr + EPS) ** 2
        return -np.log(info + EPS) / LOG2

    h_s = h2(tr_s, ss_s)
    h_t = h2(tr_t, ss_t)
    h_mix = h2(tr_s + tr_t, ss_s + 2.0 * ss_st + ss_t)
    per_class = h_mix - 0.5 * (h_s + h_t)
    valid = (n_s >= 2) & (n_t >= 2)
    n_valid = float(valid.sum())
    creda_sum = float(np.where(valid, per_class, 0.0).sum())
    loss_creda = creda_sum / max(n_valid, 1.0) if n_valid > 0 else 0.0

    # source cross entropy (host, fp64)
    zs = np.asarray(logits_s, dtype=np.float64)
    zs = zs - zs.max(axis=1, keepdims=True)
    lse = np.log(np.exp(zs).sum(axis=1))
    logp = zs - lse[:, None]
    loss_cls = -float(np.mean(logp[np.arange(N), lab]))

    # target entropy
    loss_ent = -float(np.mean(np.sum(pt * np.log(pt + EPS), axis=1)))

    total = loss_cls + LAMBDA_CREDA * loss_creda + LAMBDA_ENTROPY * loss_ent
    return np.array(total, dtype=np.float32)


def run(inputs, trace=False, repeat=1):
    """Full pipeline; returns (loss, BassKernelResults)."""
    in_maps, aux = _host_prep(**inputs)
    nc = _get_nc(repeat)
    res = run_bass_kernel_spmd(
        nc, in_maps, core_ids=list(range(NCORES)), trace=trace,
    )
    L = np.zeros((3, C, N), dtype=np.float64)
    for r in res.results:
        L += r["lout"].astype(np.float64)
    loss = _host_finish(L, aux, inputs["logits_s"])
    return loss, res


def kernel(**inputs) -> np.ndarray:
    loss, _ = run(inputs, trace=False)
    return loss
